# revision 1
# baseline (speedup 1.0000x reference)
"""GQA attention kernel for Trainium2, 8 NeuronCores.

Problem: resid [2, 2048, 1024], 16 Q heads / 8 KV groups, d_head 64, causal,
out = softmax(QK^T/8 + causal) V -> W_out + b_out.

Sharding: tensor-parallel over (batch x kv-group-pairs). Core c handles
batch b = c // 4 and kv groups {2*(c%4), 2*(c%4)+1} = 4 Q heads. Each core
computes its heads' attention and a partial output projection; the host sums
the 4 partials per batch element and adds b_out.

Per-core dataflow (fp32 storage, float32r matmuls = full PE speed at
moving-dim >= 256):
  - host passes resid[b].T so the d_model contraction lands on partitions
  - Q^T [256, S] and K^T [128, S] projections (PSUM accum over 8 d-chunks)
  - V [S, 2x65] with a ones column appended per group -> the AV matmul
    produces sum-exp for free in output row 64
  - scores computed transposed: S^T[k, q] = K @ Q^T; causality via q-start
    offset, zero-padding of exp tiles, and an upper-triangular
    multiplicative mask on diagonal tiles
  - softmax without max-subtraction (scores are O(1) by construction;
    masked lanes are exactly zero after the mask multiply)
  - U^T[e, q] += V_aug^T @ exp accumulated over k-tiles in PSUM
  - normalize: reciprocal of row 64 (VectorE), partition-broadcast
    (GpSimd), multiply into z^T (VectorE)
  - out_partial[s, d] = z^T.T @ W_out_stack accumulated over 2 e-chunks
"""

import sys

sys.path.insert(0, "/opt/trn_rl_repo")

import numpy as np

import concourse.bass as bass
import concourse.mybir as mybir
import concourse.tile as tile
from concourse import bacc
from concourse.bass_utils import run_bass_kernel_spmd
from concourse.masks import make_upper_triangular

S = 2048          # seq len
D = 1024          # d_model
E = 64            # d_head
P = 128
NC_HEADS = 4      # heads per core
NCHUNK = D // P   # 8 d_model chunks
SPAN = 512
NSPAN = S // SPAN
NKT = S // P      # 16 k tiles
F32 = mybir.dt.float32
F32R = mybir.dt.float32r
EXP = mybir.ActivationFunctionType.Exp

LAST_RESULTS = None  # stashed BassKernelResults for the test harness
_CACHED_NC = None


def _build_program():
    nc = bacc.Bacc("TRN2", target_bir_lowering=False, debug=False)

    rT_d = nc.dram_tensor("resid_t", [D, S], F32R, kind="ExternalInput")
    wq_d = nc.dram_tensor("wq", [D, 256], F32R, kind="ExternalInput")
    wk_d = nc.dram_tensor("wk", [D, 128], F32R, kind="ExternalInput")
    wv_d = nc.dram_tensor("wv", [D, 128], F32R, kind="ExternalInput")
    wo_d = nc.dram_tensor("wo", [256, D], F32R, kind="ExternalInput")
    ones_d = nc.dram_tensor("ones", [P, 1], F32R, kind="ExternalInput")
    out_d = nc.dram_tensor("out", [S, D], F32, kind="ExternalOutput")

    with tile.TileContext(nc) as tc:
        with (
            tc.tile_pool(name="persist", bufs=1) as pp,
            tc.tile_pool(name="exp", bufs=12) as ep,
            tc.tile_pool(name="zt", bufs=3) as zp,
            tc.tile_pool(name="misc", bufs=4) as mp,
            tc.tile_pool(name="ostage", bufs=4) as op,
            tc.tile_pool(name="ps_u", bufs=4, space="PSUM") as ps_u,
            tc.tile_pool(name="ps_sc", bufs=3, space="PSUM") as ps_sc,
            tc.tile_pool(name="ps_op", bufs=1, space="PSUM") as ps_op,
        ):
            # ---- load weights + transposed residual ----
            wq_sb = []
            wk_sb = []
            wv_sb = []
            for c in range(NCHUNK):
                t = pp.tile([P, 256], F32R, tag=f"wq{c}")
                nc.sync.dma_start(t[:], wq_d[c * P:(c + 1) * P, :])
                wq_sb.append(t)
                t = pp.tile([P, 128], F32R, tag=f"wk{c}")
                nc.sync.dma_start(t[:], wk_d[c * P:(c + 1) * P, :])
                wk_sb.append(t)
                t = pp.tile([P, 128], F32R, tag=f"wv{c}")
                nc.sync.dma_start(t[:], wv_d[c * P:(c + 1) * P, :])
                wv_sb.append(t)
            wo_sb = []
            for c in range(2):
                t = pp.tile([P, D], F32R, tag=f"wo{c}")
                nc.sync.dma_start(t[:], wo_d[c * P:(c + 1) * P, :])
                wo_sb.append(t)

            mask = pp.tile([P, P], F32, tag="mask")
            make_upper_triangular(nc, mask[:], val=1.0, diag=True)

            # residual chunks, DMA'd span-wise so projection accumulation
            # groups (which need all 8 d-chunks of one span) start after
            # ~2MB instead of the full 8.4MB
            rT = []
            for c in range(NCHUNK):
                t = pp.tile([P, S], F32R, tag=f"rt{c}", name=f"rt{c}")
                rT.append(t)
            for sp in range(NSPAN):
                for c in range(NCHUNK):
                    nc.sync.dma_start(
                        rT[c][:, sp * SPAN:(sp + 1) * SPAN],
                        rT_d[c * P:(c + 1) * P, sp * SPAN:(sp + 1) * SPAN])

            qT = [pp.tile([P, S], F32R, tag=f"qt{e}", name=f"qt{e}")
                  for e in range(2)]
            kT = pp.tile([P, S], F32R, tag="kt")
            vaug = [pp.tile([P, 130], F32R, tag=f"va{k}", name=f"va{k}")
                    for k in range(NKT)]

            # ---- per span: Q/K/V projection for this span, then attention.
            # Interleaving keeps PE fed with projection matmuls while ACT
            # (the exp bottleneck) works through the previous tiles. ----
            for sp in range(NSPAN):
                for eblk in range(2):
                    acc = ps_u.tile([P, SPAN], F32, tag="u", name="qacc")
                    for c in range(NCHUNK):
                        nc.tensor.matmul(
                            acc[:],
                            wq_sb[c][:, eblk * P:(eblk + 1) * P],
                            rT[c][:, sp * SPAN:(sp + 1) * SPAN],
                            start=(c == 0),
                            stop=(c == NCHUNK - 1),
                        )
                    nc.vector.tensor_copy(
                        qT[eblk][:, sp * SPAN:(sp + 1) * SPAN], acc[:])
                acc = ps_u.tile([P, SPAN], F32, tag="u", name="kacc")
                for c in range(NCHUNK):
                    nc.tensor.matmul(
                        acc[:],
                        wk_sb[c][:],
                        rT[c][:, sp * SPAN:(sp + 1) * SPAN],
                        start=(c == 0),
                        stop=(c == NCHUNK - 1),
                    )
                nc.vector.tensor_copy(kT[:, sp * SPAN:(sp + 1) * SPAN], acc[:])
                for kt in range(4 * sp, 4 * sp + 4):
                    va = vaug[kt]
                    acc = ps_sc.tile([P, SPAN], F32, tag="sc", name="vacc")
                    for c in range(NCHUNK):
                        nc.tensor.matmul(
                            acc[:, 0:128],
                            rT[c][:, kt * P:(kt + 1) * P],
                            wv_sb[c][:],
                            start=(c == 0),
                            stop=(c == NCHUNK - 1),
                        )
                    nc.vector.tensor_copy(va[:, 0:64], acc[:, 0:64])
                    nc.vector.tensor_copy(va[:, 65:129], acc[:, 64:128])
                    nc.sync.dma_start(va[:, 64:65], ones_d[:])
                    nc.sync.dma_start(va[:, 129:130], ones_d[:])

                q0 = sp * SPAN
                nkt = (q0 + SPAN) // P  # k tiles touching this span
                # head slot (g, i): local head 2g+i, stored in qT[i] rows
                # g*64:(g+1)*64 so scores lhsT/rhs share base partition g*64
                # (and g0/g1 matmuls row-pack the PE array).
                u_ps = [ps_u.tile([P, SPAN], F32, tag="u", name=f"u{j}")
                        for j in range(NC_HEADS)]
                # software pipeline: AV of k-tile kt is emitted after the
                # scores+exp of kt+1, hiding the ACT exp latency from PE
                def emit_av(batch):
                    for g, i, e_sb, kt_, off_, w_ in batch:
                        nc.tensor.matmul(
                            u_ps[2 * g + i][0:65, off_:off_ + w_],
                            vaug[kt_][:, g * 65:(g + 1) * 65],
                            e_sb[:, off_:off_ + w_],
                            start=(kt_ == 0),
                            stop=(kt_ == nkt - 1),
                            skip_group_check=True,
                        )

                pending = []
                for kt in range(nkt):
                    k0 = kt * P
                    off = max(k0 - q0, 0)
                    w = SPAN - off
                    cur = []
                    for g in range(2):
                        for i in range(2):
                            s_ps = ps_sc.tile([P, SPAN], F32, tag="sc",
                                              name=f"s{g}{i}")
                            nc.tensor.matmul(
                                s_ps[:, off:off + w],
                                kT[g * 64:(g + 1) * 64, k0:k0 + P],
                                qT[i][g * 64:(g + 1) * 64,
                                         q0 + off:q0 + off + w],
                                start=True,
                                stop=True,
                            )
                            e_sb = ep.tile([P, SPAN], F32R, tag="e",
                                           name=f"e{g}{i}")
                            nc.scalar.activation(
                                e_sb[:, off:off + w], s_ps[:, off:off + w],
                                EXP, scale=0.125,
                            )
                            if k0 >= q0:  # diagonal tile -> causal mask
                                nc.vector.tensor_mul(
                                    e_sb[:, off:off + P],
                                    e_sb[:, off:off + P].bitcast(F32),
                                    mask[:],
                                )
                            cur.append((g, i, e_sb, kt, off, w))
                    emit_av(pending)
                    pending = cur
                emit_av(pending)

                # normalize -> z^T chunks; zc[i] rows g*64 = head slot (g, i),
                # matching the host-side wo packing [h0, h2 | h1, h3]
                zc = [zp.tile([P, SPAN], F32R, tag=f"zt{c}", name=f"z{c}")
                      for c in range(2)]
                for g in range(2):
                    for i in range(2):
                        # 1/x as exp(-ln x) on ScalarE: ~4x faster than the
                        # single-partition DVE reciprocal and off its queue
                        lnt = mp.tile([1, SPAN], F32, tag="ln", name="lnt")
                        nc.scalar.activation(
                            lnt[:], u_ps[2 * g + i][64:65, :],
                            mybir.ActivationFunctionType.Ln)
                        rec = mp.tile([1, SPAN], F32, tag="rec", name="rec")
                        nc.scalar.activation(rec[:], lnt[:], EXP, scale=-1.0)
                        bc = mp.tile([64, SPAN], F32, tag="bc", name="bc")
                        nc.gpsimd.partition_broadcast(bc[:], rec[:])
                        nc.vector.tensor_mul(
                            zc[i][g * 64:(g + 1) * 64, :],
                            u_ps[2 * g + i][0:64, :],
                            bc[:],
                        )

                # output projection for this span of s
                for st in range(4):
                    s0 = q0 + st * P
                    o_sb = op.tile([P, D], F32, tag="ost")
                    for dsp in range(2):
                        o_ps = ps_op.tile([P, SPAN], F32, tag="op")
                        for ch in range(2):
                            nc.tensor.matmul(
                                o_ps[:],
                                zc[ch][:, st * P:(st + 1) * P],
                                wo_sb[ch][:, dsp * SPAN:(dsp + 1) * SPAN],
                                start=(ch == 0),
                                stop=(ch == 1),
                            )
                        nc.vector.tensor_copy(
                            o_sb[:, dsp * SPAN:(dsp + 1) * SPAN], o_ps[:])
                    nc.sync.dma_start(out_d[s0:s0 + P, :], o_sb[:])

    nc.finalize()
    return nc


def kernel(resid, W_Q, W_K, W_V, W_out, b_out):
    global LAST_RESULTS, _CACHED_NC
    resid = np.asarray(resid, np.float32)
    W_Q = np.asarray(W_Q, np.float32)
    W_K = np.asarray(W_K, np.float32)
    W_V = np.asarray(W_V, np.float32)
    W_out = np.asarray(W_out, np.float32)
    b_out = np.asarray(b_out, np.float32)

    if _CACHED_NC is None:
        _CACHED_NC = _build_program()
    nc = _CACHED_NC

    residT = [np.ascontiguousarray(resid[b].T) for b in range(2)]
    in_maps = []
    for c in range(8):
        b, q = c // 4, c % 4
        # interleaved head order [h0, h2, h1, h3]: storage slot (g, i) holds
        # local head 2g+i -> qT[i]/zc[i] rows g*64 (see _build_program)
        heads = [4 * q, 4 * q + 2, 4 * q + 1, 4 * q + 3]
        groups = [2 * q, 2 * q + 1]
        in_maps.append({
            "resid_t": residT[b],
            "wq": np.ascontiguousarray(W_Q[:, heads, :].reshape(D, 256)),
            "wk": np.ascontiguousarray(W_K[:, groups, :].reshape(D, 128)),
            "wv": np.ascontiguousarray(W_V[:, groups, :].reshape(D, 128)),
            "wo": np.ascontiguousarray(
                W_out[:, heads, :].transpose(1, 0, 2).reshape(256, D)),
            "ones": np.ones((P, 1), np.float32),
        })

    res = run_bass_kernel_spmd(nc, in_maps, core_ids=list(range(8)))
    LAST_RESULTS = res

    out = np.zeros((2, S, D), np.float32)
    for c in range(8):
        out[c // 4] += res.results[c]["out"]
    out += b_out
    return out



# revision 6
# speedup vs baseline: 1.4060x; 1.4060x over previous
"""GQA attention kernel for Trainium2, 8 NeuronCores.

Problem: resid [2, 2048, 1024], 16 Q heads / 8 KV groups, d_head 64, causal,
out = softmax(QK^T/8 + causal) V -> W_out + b_out.

Sharding: tensor-parallel over (batch x kv-group-pairs). Core c handles
batch b = c // 4 and kv groups {2*(c%4), 2*(c%4)+1} = 4 Q heads. Each core
computes its heads' attention and a partial output projection; the host sums
the 4 partials per batch element and adds b_out.

Per-core dataflow, all-bf16 operands (fp32 PSUM accumulation):
  - host passes resid[b].T (bf16) so the d_model contraction lands on
    partitions; weights pre-packed per core, bf16
  - per span (512 q): Q^T [256, S], K^T [128, S] projections; V [S, 2x65]
    with a ones column per group so the AV matmul produces sum-exp in row 64
  - scores transposed: S^T[k, q] = K @ Q^T; the two kv-groups use base
    partitions 0/64 so their matmuls row-pack the PE array and run
    concurrently; both land in one [128, 1024] two-bank PSUM tile
  - one [128, 1024] exp per k-tile on ScalarE (2-bank read amortizes the
    ~350-cycle activation pipe-fill); causal handled by q-start offset and
    an upper-triangular multiplicative mask on diagonal tiles (DVE, bf16)
  - heads processed in 2 passes per span so PSUM fits: 2 u-accumulator
    banks + 2x2 double-buffered score banks + 2 filler banks
  - projection / output-projection matmuls are emitted as "filler" units
    between attention iterations so the PE never idles (HAM stays warm)
  - normalize: sum-exp -> DVE reciprocal_approx_fast, Pool broadcast,
    DVE multiply -> z^T (bf16)
  - out_partial[s, d] = z^T.T @ W_out_stack, staged bf16, host upcasts+sums
"""

import sys

sys.path.insert(0, "/opt/trn_rl_repo")

import numpy as np
import ml_dtypes

import concourse.bass as bass
import concourse.mybir as mybir
import concourse.tile as tile
from concourse import bacc
from concourse.bass_utils import run_bass_kernel_spmd
from concourse.masks import make_upper_triangular

S = 2048          # seq len
D = 1024          # d_model
E = 64            # d_head
P = 128
NCHUNK = D // P   # 8 d_model chunks
SPAN = 512
NSPAN = S // SPAN
NKT = S // P      # 16 k tiles
F32 = mybir.dt.float32
BF16 = mybir.dt.bfloat16
EXP = mybir.ActivationFunctionType.Exp

LAST_RESULTS = None  # stashed BassKernelResults for the test harness
_CACHED_NC = None


def _build_program(debug=False):
    nc = bacc.Bacc("TRN2", target_bir_lowering=False, debug=False)
    dbg = {}

    def dbg_out(name, shape, dt):
        dbg[name] = nc.dram_tensor(name, shape, dt, kind="ExternalOutput")
        return dbg[name]

    rT_d = nc.dram_tensor("resid_t", [D, S], BF16, kind="ExternalInput")
    wq_d = nc.dram_tensor("wq", [D, 256], BF16, kind="ExternalInput")
    wk_d = nc.dram_tensor("wk", [D, 128], BF16, kind="ExternalInput")
    wv_d = nc.dram_tensor("wv", [D, 128], BF16, kind="ExternalInput")
    wo_d = nc.dram_tensor("wo", [256, D], BF16, kind="ExternalInput")
    out_d = nc.dram_tensor("out", [S, D], BF16, kind="ExternalOutput")

    with tile.TileContext(nc) as tc:
        with (
            tc.tile_pool(name="persist", bufs=1) as pp,
            tc.tile_pool(name="exp", bufs=4) as ep,
            tc.tile_pool(name="norm", bufs=3) as mp,
            tc.tile_pool(name="ostage", bufs=3) as op,
            tc.tile_pool(name="ps_sc", bufs=2, space="PSUM") as ps_sc,
            tc.tile_pool(name="ps_u", bufs=2, space="PSUM") as ps_u,
            tc.tile_pool(name="ps_f", bufs=2, space="PSUM") as ps_f,
        ):
            # ---- weights ----
            wq_sb = []
            wk_sb = []
            wv_sb = []
            for c in range(NCHUNK):
                t = pp.tile([P, 256], BF16, tag=f"wq{c}")
                nc.sync.dma_start(t[:], wq_d[c * P:(c + 1) * P, :])
                wq_sb.append(t)
                t = pp.tile([P, 128], BF16, tag=f"wk{c}")
                nc.sync.dma_start(t[:], wk_d[c * P:(c + 1) * P, :])
                wk_sb.append(t)
                t = pp.tile([P, 128], BF16, tag=f"wv{c}")
                nc.sync.dma_start(t[:], wv_d[c * P:(c + 1) * P, :])
                wv_sb.append(t)

            mask = pp.tile([P, P], BF16, tag="mask")
            make_upper_triangular(nc, mask[:], val=1.0, diag=True)

            # residual chunks, span-major so span 0 lands first
            rT = [pp.tile([P, S], BF16, tag=f"rt{c}", name=f"rt{c}")
                  for c in range(NCHUNK)]
            for sp in range(NSPAN):
                for c in range(NCHUNK):
                    nc.sync.dma_start(
                        rT[c][:, sp * SPAN:(sp + 1) * SPAN],
                        rT_d[c * P:(c + 1) * P, sp * SPAN:(sp + 1) * SPAN])
                if sp == 0:
                    wo_sb = []
                    for c in range(2):
                        t = pp.tile([P, D], BF16, tag=f"wo{c}")
                        nc.sync.dma_start(t[:], wo_d[c * P:(c + 1) * P, :])
                        wo_sb.append(t)

            qT = [pp.tile([P, S], BF16, tag=f"qt{e}", name=f"qt{e}")
                  for e in range(2)]
            kT = pp.tile([P, S], BF16, tag="kt")
            vaug = [pp.tile([P, 130], BF16, tag=f"va{k}", name=f"va{k}")
                    for k in range(NKT)]
            for k in range(NKT):
                nc.gpsimd.memset(vaug[k][:, 64:65], 1.0)
                nc.gpsimd.memset(vaug[k][:, 129:130], 1.0)
            # z^T per (span, pass): rows g*64..g*64+63 = head slot (g, i)
            zc = [[pp.tile([P, SPAN], BF16, tag=f"zc{sp}{i}", name=f"z{sp}{i}")
                   for i in range(2)] for sp in range(NSPAN)]

            # ---- filler units: projection + output-projection matmul
            # groups run between attention iterations on 2 spare PSUM banks
            # so the PE never goes idle while ScalarE works through exps ----
            filler = []

            def q_proj_unit(sp, eblk):
                def go():
                    acc = ps_f.tile([P, SPAN], F32, tag="f", name="qacc")
                    for c in range(NCHUNK):
                        nc.tensor.matmul(
                            acc[:],
                            wq_sb[c][:, eblk * P:(eblk + 1) * P],
                            rT[c][:, sp * SPAN:(sp + 1) * SPAN],
                            start=(c == 0),
                            stop=(c == NCHUNK - 1),
                        )
                    nc.vector.tensor_copy(
                        qT[eblk][:, sp * SPAN:(sp + 1) * SPAN], acc[:])
                return go

            def k_proj_unit(sp):
                def go():
                    acc = ps_f.tile([P, SPAN], F32, tag="f", name="kacc")
                    for c in range(NCHUNK):
                        nc.tensor.matmul(
                            acc[:],
                            wk_sb[c][:],
                            rT[c][:, sp * SPAN:(sp + 1) * SPAN],
                            start=(c == 0),
                            stop=(c == NCHUNK - 1),
                        )
                    nc.vector.tensor_copy(
                        kT[:, sp * SPAN:(sp + 1) * SPAN], acc[:])
                return go

            def v_proj_unit(kt):
                def go():
                    acc = ps_f.tile([P, SPAN], F32, tag="f", name="vacc")
                    for c in range(NCHUNK):
                        nc.tensor.matmul(
                            acc[:, 0:128],
                            rT[c][:, kt * P:(kt + 1) * P],
                            wv_sb[c][:],
                            start=(c == 0),
                            stop=(c == NCHUNK - 1),
                        )
                    nc.vector.tensor_copy(vaug[kt][:, 0:64], acc[:, 0:64])
                    nc.vector.tensor_copy(vaug[kt][:, 65:129], acc[:, 64:128])
                return go

            def op_unit(sp, st):
                def go():
                    s0 = sp * SPAN + st * P
                    o_sb = op.tile([P, D], BF16, tag="ost")
                    for dsp in range(2):
                        o_ps = ps_f.tile([P, SPAN], F32, tag="f", name="ops")
                        for ch in range(2):
                            nc.tensor.matmul(
                                o_ps[:],
                                zc[sp][ch][:, st * P:(st + 1) * P],
                                wo_sb[ch][:, dsp * SPAN:(dsp + 1) * SPAN],
                                start=(ch == 0),
                                stop=(ch == 1),
                            )
                        nc.vector.tensor_copy(
                            o_sb[:, dsp * SPAN:(dsp + 1) * SPAN], o_ps[:])
                    nc.sync.dma_start(out_d[s0:s0 + P, :], o_sb[:])
                return go

            def pump(n):
                for _ in range(min(n, len(filler))):
                    filler.pop(0)()

            def proj_units(sp):
                u = [q_proj_unit(sp, 0), q_proj_unit(sp, 1), k_proj_unit(sp)]
                u += [v_proj_unit(kt) for kt in range(4 * sp, 4 * sp + 4)]
                return u

            # span 0 projections run up front
            for f in proj_units(0):
                f()

            for sp in range(NSPAN):
                q0 = sp * SPAN
                nkt = (q0 + SPAN) // P
                if sp + 1 < NSPAN:
                    filler.extend(proj_units(sp + 1))
                for ip in range(2):
                    u_ps = [ps_u.tile([65, SPAN], F32, tag="u", name=f"u{g}")
                            for g in range(2)]

                    def emit_av(b):
                        kt_, off_, w_, e_ = b
                        for g in range(2):
                            nc.tensor.matmul(
                                u_ps[g][0:65, off_:off_ + w_],
                                vaug[kt_][:, g * 65:(g + 1) * 65],
                                e_[:, g * 512 + off_:g * 512 + off_ + w_],
                                start=(kt_ == 0),
                                stop=(kt_ == nkt - 1),
                                skip_group_check=True,
                            )

                    pending = None
                    for kt in range(nkt):
                        k0 = kt * P
                        off = max(k0 - q0, 0)
                        w = SPAN - off
                        s_ps = ps_sc.tile([P, 2 * SPAN], F32, tag="sc",
                                          name="sps")
                        for g in range(2):
                            nc.tensor.matmul(
                                s_ps[:, g * 512 + off:g * 512 + off + w],
                                kT[g * 64:(g + 1) * 64, k0:k0 + P],
                                qT[ip][g * 64:(g + 1) * 64,
                                       q0 + off:q0 + off + w],
                                start=True,
                                stop=True,
                            )
                        e_sb = ep.tile([P, 2 * SPAN], BF16, tag="e", name="e")
                        nc.scalar.activation(
                            e_sb[:, off:2 * SPAN], s_ps[:, off:2 * SPAN],
                            EXP, scale=0.125,
                        )
                        if k0 >= q0:  # diagonal tile -> causal mask
                            for g in range(2):
                                nc.vector.tensor_mul(
                                    e_sb[:, g * 512 + off:g * 512 + off + P],
                                    e_sb[:, g * 512 + off:g * 512 + off + P],
                                    mask[:],
                                )
                        if debug and (sp, ip, kt) in ((0, 0, 0), (1, 0, 2)):
                            t = dbg_out(f"d_e_{sp}_{ip}_{kt}",
                                        [P, 2 * SPAN], BF16)
                            nc.sync.dma_start(t[:], e_sb[:])
                        if pending is not None:
                            emit_av(pending)
                        pending = (kt, off, w, e_sb)
                        pump(1)
                    emit_av(pending)
                    if debug and sp == 0 and ip == 0:
                        for g in range(2):
                            us = pp.tile([65, SPAN], F32, tag=f"dbgu{g}")
                            nc.vector.tensor_copy(us[:], u_ps[g][:])
                            t = dbg_out(f"d_u_{g}", [65, SPAN], F32)
                            nc.sync.dma_start(t[:], us[:])

                    # normalize this pass -> z^T slabs
                    for g in range(2):
                        # standard-op copy remaps partition 64 -> 0; the
                        # custom-DVE reciprocal needs lane-aligned operands
                        row = mp.tile([1, SPAN], F32, tag="row", name="row")
                        nc.vector.tensor_copy(row[:], u_ps[g][64:65, :])
                        rec = mp.tile([1, SPAN], F32, tag="rec", name="rec")
                        nc.vector.reciprocal_approx_fast(rec[:], row[:])
                        bc = mp.tile([64, SPAN], F32, tag="bc", name="bc")
                        nc.gpsimd.partition_broadcast(bc[:], rec[:])
                        nc.vector.tensor_mul(
                            zc[sp][ip][g * 64:(g + 1) * 64, :],
                            u_ps[g][0:64, :],
                            bc[:],
                        )
                        if debug and sp == 0 and ip == 0:
                            rs = pp.tile([1, SPAN], F32, tag=f"dbgr{g}")
                            nc.vector.tensor_copy(rs[:], rec[:])
                            t = dbg_out(f"d_rec_{g}", [1, SPAN], F32)
                            nc.sync.dma_start(t[:], rs[:])
                if debug and sp == 0:
                    for i in range(2):
                        t = dbg_out(f"d_zc_{i}", [P, SPAN], BF16)
                        nc.sync.dma_start(t[:], zc[0][i][:])
                filler.extend(op_unit(sp, st) for st in range(4))
            pump(len(filler))
            if debug:
                for nm, ap in (("d_mask", mask), ("d_kT", kT),
                               ("d_qT0", qT[0]), ("d_qT1", qT[1]),
                               ("d_va0", vaug[0]), ("d_va5", vaug[5])):
                    t = dbg_out(nm, list(ap.shape), BF16)
                    nc.sync.dma_start(t[:], ap[:])

    nc.finalize()
    return nc


def kernel(resid, W_Q, W_K, W_V, W_out, b_out):
    global LAST_RESULTS, _CACHED_NC
    resid = np.asarray(resid, np.float32)
    W_Q = np.asarray(W_Q, np.float32)
    W_K = np.asarray(W_K, np.float32)
    W_V = np.asarray(W_V, np.float32)
    W_out = np.asarray(W_out, np.float32)
    b_out = np.asarray(b_out, np.float32)
    bf16 = ml_dtypes.bfloat16

    if _CACHED_NC is None:
        _CACHED_NC = _build_program()
    nc = _CACHED_NC

    residT = [np.ascontiguousarray(resid[b].T).astype(bf16) for b in range(2)]
    in_maps = []
    for c in range(8):
        b, q = c // 4, c % 4
        # interleaved head order [h0, h2, h1, h3]: storage slot (g, i) holds
        # local head 2g+i -> qT[i]/zc[i] rows g*64 (see _build_program)
        heads = [4 * q, 4 * q + 2, 4 * q + 1, 4 * q + 3]
        groups = [2 * q, 2 * q + 1]
        in_maps.append({
            "resid_t": residT[b],
            "wq": np.ascontiguousarray(
                W_Q[:, heads, :].reshape(D, 256)).astype(bf16),
            "wk": np.ascontiguousarray(
                W_K[:, groups, :].reshape(D, 128)).astype(bf16),
            "wv": np.ascontiguousarray(
                W_V[:, groups, :].reshape(D, 128)).astype(bf16),
            "wo": np.ascontiguousarray(
                W_out[:, heads, :].transpose(1, 0, 2).reshape(256, D)
            ).astype(bf16),
        })

    res = run_bass_kernel_spmd(nc, in_maps, core_ids=list(range(8)))
    LAST_RESULTS = res

    out = np.zeros((2, S, D), np.float32)
    for c in range(8):
        out[c // 4] += np.asarray(res.results[c]["out"], np.float32)
    out += b_out
    return out


# revision 7
# speedup vs baseline: 1.5872x; 1.1289x over previous
"""GQA attention kernel for Trainium2, 8 NeuronCores.

Problem: resid [2, 2048, 1024], 16 Q heads / 8 KV groups, d_head 64, causal,
out = softmax(QK^T/8 + causal) V -> W_out + b_out.

Sharding: tensor-parallel over (batch x kv-group-pairs). Core c handles
batch b = c // 4 and kv groups {2*(c%4), 2*(c%4)+1} = 4 Q heads. Each core
computes its heads' attention and a partial output projection; the host sums
the 4 partials per batch element and adds b_out.

Per-core dataflow, all-bf16 operands (fp32 PSUM accumulation):
  - host passes resid[b].T (bf16) so the d_model contraction lands on
    partitions; weights pre-packed per core, bf16
  - per span (512 q): Q^T [256, S], K^T [128, S] projections; V [S, 2x65]
    with a ones column per group so the AV matmul produces sum-exp in row 64
  - scores transposed: S^T[k, q] = K @ Q^T; the two kv-groups use base
    partitions 0/64 so their matmuls row-pack the PE array and run
    concurrently; both land in one [128, 1024] two-bank PSUM tile
  - one [128, 1024] exp per k-tile on ScalarE (2-bank read amortizes the
    ~350-cycle activation pipe-fill); causal handled by q-start offset and
    an upper-triangular multiplicative mask on diagonal tiles (DVE, bf16)
  - heads processed in 2 passes per span so PSUM fits: 2 u-accumulator
    banks + 2x2 double-buffered score banks + 2 filler banks
  - projection / output-projection matmuls are emitted as "filler" units
    between attention iterations so the PE never idles (HAM stays warm)
  - normalize: sum-exp -> DVE reciprocal_approx_fast, Pool broadcast,
    DVE multiply -> z^T (bf16)
  - out_partial[s, d] = z^T.T @ W_out_stack, staged bf16, host upcasts+sums
"""

import sys

sys.path.insert(0, "/opt/trn_rl_repo")

import numpy as np
import ml_dtypes

import concourse.bass as bass
import concourse.mybir as mybir
import concourse.tile as tile
from concourse import bacc
from concourse.bass_utils import run_bass_kernel_spmd
from concourse.masks import make_upper_triangular

S = 2048          # seq len
D = 1024          # d_model
E = 64            # d_head
P = 128
NCHUNK = D // P   # 8 d_model chunks
SPAN = 512
NSPAN = S // SPAN
NKT = S // P      # 16 k tiles
F32 = mybir.dt.float32
BF16 = mybir.dt.bfloat16
EXP = mybir.ActivationFunctionType.Exp

LAST_RESULTS = None  # stashed BassKernelResults for the test harness
_CACHED_NC = None


def _build_program(debug=False):
    nc = bacc.Bacc("TRN2", target_bir_lowering=False, debug=False)
    dbg = {}

    def dbg_out(name, shape, dt):
        dbg[name] = nc.dram_tensor(name, shape, dt, kind="ExternalOutput")
        return dbg[name]

    rT_d = nc.dram_tensor("resid_t", [D, S], BF16, kind="ExternalInput")
    wq_d = nc.dram_tensor("wq", [D, 256], BF16, kind="ExternalInput")
    wk_d = nc.dram_tensor("wk", [D, 128], BF16, kind="ExternalInput")
    wv_d = nc.dram_tensor("wv", [D, 128], BF16, kind="ExternalInput")
    wo_d = nc.dram_tensor("wo", [256, D], BF16, kind="ExternalInput")
    out_d = nc.dram_tensor("out", [S, D], BF16, kind="ExternalOutput")

    with tile.TileContext(nc) as tc:
        with (
            tc.tile_pool(name="persist", bufs=1) as pp,
            tc.tile_pool(name="exp", bufs=4) as ep,
            tc.tile_pool(name="norm", bufs=3) as mp,
            tc.tile_pool(name="ostage", bufs=3) as op,
            tc.tile_pool(name="ps_sc", bufs=2, space="PSUM") as ps_sc,
            tc.tile_pool(name="ps_u", bufs=2, space="PSUM") as ps_u,
            tc.tile_pool(name="ps_f", bufs=2, space="PSUM") as ps_f,
        ):
            # ---- weights, split across the two HWDGE queues ----
            wq_sb = []
            wk_sb = []
            wv_sb = []
            for c in range(NCHUNK):
                t = pp.tile([P, 128], BF16, tag=f"wk{c}")
                nc.sync.dma_start(t[:], wk_d[c * P:(c + 1) * P, :])
                wk_sb.append(t)
                t = pp.tile([P, 128], BF16, tag=f"wv{c}")
                nc.sync.dma_start(t[:], wv_d[c * P:(c + 1) * P, :])
                wv_sb.append(t)
                t = pp.tile([P, 256], BF16, tag=f"wq{c}")
                nc.scalar.dma_start(t[:], wq_d[c * P:(c + 1) * P, :])
                wq_sb.append(t)

            mask = pp.tile([P, P], BF16, tag="mask")
            make_upper_triangular(nc, mask[:], val=1.0, diag=True)

            # residual: span 0 first (both queues), then one 3KB-line DMA
            # per chunk for spans 1-3
            rT = [pp.tile([P, S], BF16, tag=f"rt{c}", name=f"rt{c}")
                  for c in range(NCHUNK)]
            for c in range(NCHUNK):
                eng = nc.sync if c % 2 == 0 else nc.scalar
                eng.dma_start(rT[c][:, 0:SPAN], rT_d[c * P:(c + 1) * P, 0:SPAN])
            wo_sb = []
            for c in range(2):
                t = pp.tile([P, D], BF16, tag=f"wo{c}")
                nc.scalar.dma_start(t[:], wo_d[c * P:(c + 1) * P, :])
                wo_sb.append(t)
            for c in range(NCHUNK):
                eng = nc.sync if c % 2 == 0 else nc.scalar
                eng.dma_start(rT[c][:, SPAN:S], rT_d[c * P:(c + 1) * P, SPAN:S])

            qT = [pp.tile([P, S], BF16, tag=f"qt{e}", name=f"qt{e}")
                  for e in range(2)]
            kT = pp.tile([P, S], BF16, tag="kt")
            vaug = [pp.tile([P, 130], BF16, tag=f"va{k}", name=f"va{k}")
                    for k in range(NKT)]
            for k in range(NKT):
                nc.gpsimd.memset(vaug[k][:, 64:65], 1.0)
                nc.gpsimd.memset(vaug[k][:, 129:130], 1.0)
            # z^T per (span, pass): rows g*64..g*64+63 = head slot (g, i)
            zc = [[pp.tile([P, SPAN], BF16, tag=f"zc{sp}{i}", name=f"z{sp}{i}")
                   for i in range(2)] for sp in range(NSPAN)]

            # ---- filler units: projection + output-projection matmul
            # groups run between attention iterations on 2 spare PSUM banks
            # so the PE never goes idle while ScalarE works through exps ----
            filler = []
            op_filler = []

            def q_proj_unit(sp, eblk):
                def go():
                    acc = ps_f.tile([P, SPAN], F32, tag="f", name="qacc")
                    for c in range(NCHUNK):
                        nc.tensor.matmul(
                            acc[:],
                            wq_sb[c][:, eblk * P:(eblk + 1) * P],
                            rT[c][:, sp * SPAN:(sp + 1) * SPAN],
                            start=(c == 0),
                            stop=(c == NCHUNK - 1),
                        )
                    nc.vector.tensor_copy(
                        qT[eblk][:, sp * SPAN:(sp + 1) * SPAN], acc[:])
                return go

            def k_proj_unit(sp):
                def go():
                    acc = ps_f.tile([P, SPAN], F32, tag="f", name="kacc")
                    for c in range(NCHUNK):
                        nc.tensor.matmul(
                            acc[:],
                            wk_sb[c][:],
                            rT[c][:, sp * SPAN:(sp + 1) * SPAN],
                            start=(c == 0),
                            stop=(c == NCHUNK - 1),
                        )
                    nc.vector.tensor_copy(
                        kT[:, sp * SPAN:(sp + 1) * SPAN], acc[:])
                return go

            def v_proj_unit(kt):
                def go():
                    acc = ps_f.tile([P, SPAN], F32, tag="f", name="vacc")
                    for c in range(NCHUNK):
                        nc.tensor.matmul(
                            acc[:, 0:128],
                            rT[c][:, kt * P:(kt + 1) * P],
                            wv_sb[c][:],
                            start=(c == 0),
                            stop=(c == NCHUNK - 1),
                        )
                    nc.vector.tensor_copy(vaug[kt][:, 0:64], acc[:, 0:64])
                    nc.vector.tensor_copy(vaug[kt][:, 65:129], acc[:, 64:128])
                return go

            def op_unit(sp, st):
                def go():
                    s0 = sp * SPAN + st * P
                    o_sb = op.tile([P, D], BF16, tag="ost")
                    for dsp in range(2):
                        o_ps = ps_f.tile([P, SPAN], F32, tag="f", name="ops")
                        for ch in range(2):
                            nc.tensor.matmul(
                                o_ps[:],
                                zc[sp][ch][:, st * P:(st + 1) * P],
                                wo_sb[ch][:, dsp * SPAN:(dsp + 1) * SPAN],
                                start=(ch == 0),
                                stop=(ch == 1),
                            )
                        nc.vector.tensor_copy(
                            o_sb[:, dsp * SPAN:(dsp + 1) * SPAN], o_ps[:])
                    nc.sync.dma_start(out_d[s0:s0 + P, :], o_sb[:])
                return go

            def pump(n, ops_ok=False):
                for _ in range(n):
                    if filler:
                        filler.pop(0)()
                    elif ops_ok and op_filler:
                        op_filler.pop(0)()
                    else:
                        break

            def proj_units(sp):
                u = [q_proj_unit(sp, 0), q_proj_unit(sp, 1), k_proj_unit(sp)]
                u += [v_proj_unit(kt) for kt in range(4 * sp, 4 * sp + 4)]
                return u

            # span 0 projections run up front
            for f in proj_units(0):
                f()

            for sp in range(NSPAN):
                q0 = sp * SPAN
                nkt = (q0 + SPAN) // P
                if sp + 1 < NSPAN:
                    filler.extend(proj_units(sp + 1))
                for ip in range(2):
                    u_ps = [ps_u.tile([65, SPAN], F32, tag="u", name=f"u{g}")
                            for g in range(2)]

                    def emit_av(b):
                        kt_, off_, w_, e_ = b
                        for g in range(2):
                            nc.tensor.matmul(
                                u_ps[g][0:65, off_:off_ + w_],
                                vaug[kt_][:, g * 65:(g + 1) * 65],
                                e_[:, g * 512 + off_:g * 512 + off_ + w_],
                                start=(kt_ == 0),
                                stop=(kt_ == nkt - 1),
                                skip_group_check=True,
                            )

                    pending = None
                    for kt in range(nkt):
                        k0 = kt * P
                        off = max(k0 - q0, 0)
                        w = SPAN - off
                        s_ps = ps_sc.tile([P, 2 * SPAN], F32, tag="sc",
                                          name="sps")
                        for g in range(2):
                            nc.tensor.matmul(
                                s_ps[:, g * 512 + off:g * 512 + off + w],
                                kT[g * 64:(g + 1) * 64, k0:k0 + P],
                                qT[ip][g * 64:(g + 1) * 64,
                                       q0 + off:q0 + off + w],
                                start=True,
                                stop=True,
                            )
                        e_sb = ep.tile([P, 2 * SPAN], BF16, tag="e", name="e")
                        nc.scalar.activation(
                            e_sb[:, off:2 * SPAN], s_ps[:, off:2 * SPAN],
                            EXP, scale=0.125,
                        )
                        if k0 >= q0:  # diagonal tile -> causal mask
                            for g in range(2):
                                nc.vector.tensor_mul(
                                    e_sb[:, g * 512 + off:g * 512 + off + P],
                                    e_sb[:, g * 512 + off:g * 512 + off + P],
                                    mask[:],
                                )
                        if debug and (sp, ip, kt) in ((0, 0, 0), (1, 0, 2)):
                            t = dbg_out(f"d_e_{sp}_{ip}_{kt}",
                                        [P, 2 * SPAN], BF16)
                            nc.sync.dma_start(t[:], e_sb[:])
                        if pending is not None:
                            emit_av(pending)
                        pending = (kt, off, w, e_sb)
                        pump(1, ops_ok=(sp == NSPAN - 1))
                    emit_av(pending)
                    if debug and sp == 0 and ip == 0:
                        for g in range(2):
                            us = pp.tile([65, SPAN], F32, tag=f"dbgu{g}")
                            nc.vector.tensor_copy(us[:], u_ps[g][:])
                            t = dbg_out(f"d_u_{g}", [65, SPAN], F32)
                            nc.sync.dma_start(t[:], us[:])

                    # normalize this pass -> z^T slabs
                    for g in range(2):
                        # standard-op copy remaps partition 64 -> 0; the
                        # custom-DVE reciprocal needs lane-aligned operands
                        row = mp.tile([1, SPAN], F32, tag="row", name="row")
                        nc.vector.tensor_copy(row[:], u_ps[g][64:65, :])
                        rec = mp.tile([1, SPAN], F32, tag="rec", name="rec")
                        nc.vector.reciprocal_approx_fast(rec[:], row[:])
                        bc = mp.tile([64, SPAN], F32, tag="bc", name="bc")
                        nc.gpsimd.partition_broadcast(bc[:], rec[:])
                        nc.vector.tensor_mul(
                            zc[sp][ip][g * 64:(g + 1) * 64, :],
                            u_ps[g][0:64, :],
                            bc[:],
                        )
                        if debug and sp == 0 and ip == 0:
                            rs = pp.tile([1, SPAN], F32, tag=f"dbgr{g}")
                            nc.vector.tensor_copy(rs[:], rec[:])
                            t = dbg_out(f"d_rec_{g}", [1, SPAN], F32)
                            nc.sync.dma_start(t[:], rs[:])
                if debug and sp == 0:
                    for i in range(2):
                        t = dbg_out(f"d_zc_{i}", [P, SPAN], BF16)
                        nc.sync.dma_start(t[:], zc[0][i][:])
                op_filler.extend(op_unit(sp, st) for st in range(4))
            pump(len(filler) + len(op_filler), ops_ok=True)
            if debug:
                for nm, ap in (("d_mask", mask), ("d_kT", kT),
                               ("d_qT0", qT[0]), ("d_qT1", qT[1]),
                               ("d_va0", vaug[0]), ("d_va5", vaug[5])):
                    t = dbg_out(nm, list(ap.shape), BF16)
                    nc.sync.dma_start(t[:], ap[:])

    nc.finalize()
    return nc


def kernel(resid, W_Q, W_K, W_V, W_out, b_out):
    global LAST_RESULTS, _CACHED_NC
    resid = np.asarray(resid, np.float32)
    W_Q = np.asarray(W_Q, np.float32)
    W_K = np.asarray(W_K, np.float32)
    W_V = np.asarray(W_V, np.float32)
    W_out = np.asarray(W_out, np.float32)
    b_out = np.asarray(b_out, np.float32)
    bf16 = ml_dtypes.bfloat16

    if _CACHED_NC is None:
        _CACHED_NC = _build_program()
    nc = _CACHED_NC

    residT = [np.ascontiguousarray(resid[b].T).astype(bf16) for b in range(2)]
    in_maps = []
    for c in range(8):
        b, q = c // 4, c % 4
        # interleaved head order [h0, h2, h1, h3]: storage slot (g, i) holds
        # local head 2g+i -> qT[i]/zc[i] rows g*64 (see _build_program)
        heads = [4 * q, 4 * q + 2, 4 * q + 1, 4 * q + 3]
        groups = [2 * q, 2 * q + 1]
        in_maps.append({
            "resid_t": residT[b],
            "wq": np.ascontiguousarray(
                W_Q[:, heads, :].reshape(D, 256)).astype(bf16),
            "wk": np.ascontiguousarray(
                W_K[:, groups, :].reshape(D, 128)).astype(bf16),
            "wv": np.ascontiguousarray(
                W_V[:, groups, :].reshape(D, 128)).astype(bf16),
            "wo": np.ascontiguousarray(
                W_out[:, heads, :].transpose(1, 0, 2).reshape(256, D)
            ).astype(bf16),
        })

    res = run_bass_kernel_spmd(nc, in_maps, core_ids=list(range(8)))
    LAST_RESULTS = res

    out = np.zeros((2, S, D), np.float32)
    for c in range(8):
        out[c // 4] += np.asarray(res.results[c]["out"], np.float32)
    out += b_out
    return out


# revision 10
# speedup vs baseline: 1.6922x; 1.0662x over previous
"""GQA attention kernel for Trainium2, 8 NeuronCores.

Problem: resid [2, 2048, 1024], 16 Q heads / 8 KV groups, d_head 64, causal,
out = softmax(QK^T/8 + causal) V -> W_out + b_out.

Sharding: tensor-parallel over (batch x kv-group-pairs). Core c handles
batch b = c // 4 and kv groups {2*(c%4), 2*(c%4)+1} = 4 Q heads. Each core
computes its heads' attention and a partial output projection; the host sums
the 4 partials per batch element and adds b_out.

Per-core dataflow, all-bf16 operands (fp32 PSUM accumulation):
  - host passes resid[b].T (bf16) so the d_model contraction lands on
    partitions; weights pre-packed per core, bf16
  - per span (512 q): Q^T [256, S], K^T [128, S] projections; V [S, 2x65]
    with a ones column per group so the AV matmul produces sum-exp in row 64
  - scores transposed: S^T[k, q] = K @ Q^T; the two kv-groups use base
    partitions 0/64 so their matmuls row-pack the PE array and run
    concurrently; both land in one [128, 1024] two-bank PSUM tile
  - one [128, 1024] exp per k-tile on ScalarE (2-bank read amortizes the
    ~350-cycle activation pipe-fill); causal handled by q-start offset and
    an upper-triangular multiplicative mask on diagonal tiles (DVE, bf16)
  - heads processed in 2 passes per span so PSUM fits: 2 u-accumulator
    banks + 2x2 double-buffered score banks + 2 filler banks
  - projection / output-projection matmuls are emitted as "filler" units
    between attention iterations so the PE never idles (HAM stays warm)
  - normalize: sum-exp -> DVE reciprocal_approx_fast, Pool broadcast,
    DVE multiply -> z^T (bf16)
  - out_partial[s, d] = z^T.T @ W_out_stack, staged bf16, host upcasts+sums
"""

import sys

sys.path.insert(0, "/opt/trn_rl_repo")

import numpy as np
import ml_dtypes

import concourse.bass as bass
import concourse.mybir as mybir
import concourse.tile as tile
from concourse import bacc
from concourse.bass_utils import run_bass_kernel_spmd
from concourse.masks import make_upper_triangular

S = 2048          # seq len
D = 1024          # d_model
E = 64            # d_head
P = 128
NCHUNK = D // P   # 8 d_model chunks
SPAN = 512
NSPAN = S // SPAN
NKT = S // P      # 16 k tiles
F32 = mybir.dt.float32
BF16 = mybir.dt.bfloat16
EXP = mybir.ActivationFunctionType.Exp

LAST_RESULTS = None  # stashed BassKernelResults for the test harness
_CACHED_NC = None


def _build_program(debug=False):
    nc = bacc.Bacc("TRN2", target_bir_lowering=False, debug=False)
    dbg = {}

    def dbg_out(name, shape, dt):
        dbg[name] = nc.dram_tensor(name, shape, dt, kind="ExternalOutput")
        return dbg[name]

    rT_d = nc.dram_tensor("resid_t", [D, S], BF16, kind="ExternalInput")
    wp_d = nc.dram_tensor("wpack", [P, 6144], BF16, kind="ExternalInput")
    out_d = nc.dram_tensor("out", [S, D], BF16, kind="ExternalOutput")

    with tile.TileContext(nc) as tc:
        with (
            tc.tile_pool(name="persist", bufs=1) as pp,
            tc.tile_pool(name="exp", bufs=4) as ep,
            tc.tile_pool(name="norm", bufs=3) as mp,
            tc.tile_pool(name="ostage", bufs=3) as op,
            tc.tile_pool(name="ps_sc", bufs=2, space="PSUM") as ps_sc,
            tc.tile_pool(name="ps_u", bufs=2, space="PSUM") as ps_u,
            tc.tile_pool(name="ps_f", bufs=2, space="PSUM") as ps_f,
        ):
            # ---- one packed weight DMA; queue-issue time is per dma_start,
            # so batching all weights into a single [128, 6144] transfer gets
            # the PE fed ~15us earlier ----
            wp = pp.tile([P, 6144], BF16, tag="wp")
            nc.sync.dma_start(wp[:], wp_d[:, :])
            wk_sb = [wp[:, c * 512:c * 512 + 128] for c in range(NCHUNK)]
            wv_sb = [wp[:, c * 512 + 128:c * 512 + 256] for c in range(NCHUNK)]
            wq_sb = [wp[:, c * 512 + 256:c * 512 + 512] for c in range(NCHUNK)]
            wo_sb = [wp[:, 4096 + c * D:4096 + (c + 1) * D] for c in range(2)]

            mask = pp.tile([P, P], BF16, tag="mask")
            make_upper_triangular(nc, mask[:], val=1.0, diag=True)

            # residual: span 0 first (both queues), then one 3KB-line DMA
            # per chunk for spans 1-3
            rT = [pp.tile([P, S], BF16, tag=f"rt{c}", name=f"rt{c}")
                  for c in range(NCHUNK)]
            for c in range(NCHUNK):
                eng = nc.scalar if c % 2 == 0 else nc.sync
                eng.dma_start(rT[c][:, 0:SPAN], rT_d[c * P:(c + 1) * P, 0:SPAN])
            for c in range(NCHUNK):
                eng = nc.scalar if c % 2 == 0 else nc.sync
                eng.dma_start(rT[c][:, SPAN:S], rT_d[c * P:(c + 1) * P, SPAN:S])

            qT = [pp.tile([P, S], BF16, tag=f"qt{e}", name=f"qt{e}")
                  for e in range(2)]
            kT = pp.tile([P, S], BF16, tag="kt")
            vaug = [pp.tile([P, 130], BF16, tag=f"va{k}", name=f"va{k}")
                    for k in range(NKT)]
            for k in range(NKT):
                nc.gpsimd.memset(vaug[k][:, 64:65], 1.0)
                nc.gpsimd.memset(vaug[k][:, 129:130], 1.0)
            # z^T per (span, pass): rows g*64..g*64+63 = head slot (g, i)
            zc = [[pp.tile([P, SPAN], BF16, tag=f"zc{sp}{i}", name=f"z{sp}{i}")
                   for i in range(2)] for sp in range(NSPAN)]

            # ---- filler units: projection + output-projection matmul
            # groups run between attention iterations on 2 spare PSUM banks
            # so the PE never goes idle while ScalarE works through exps ----
            filler = []
            op_filler = []

            def _chain_units(lhs_of, sp, dst, n_half=4):
                # an 8-chunk accumulation split into two pump units that
                # share one PSUM slot (finer PE interleave granularity)
                cell = {}

                def go_a():
                    acc = ps_f.tile([P, SPAN], F32, tag="f", name="pa")
                    cell['acc'] = acc
                    for c in range(n_half):
                        nc.tensor.matmul(
                            cell['acc'][:],
                            lhs_of(c),
                            rT[c][:, sp * SPAN:(sp + 1) * SPAN],
                            start=(c == 0),
                            stop=False,
                            skip_group_check=True,
                        )

                def go_b():
                    acc = cell['acc']
                    for c in range(n_half, NCHUNK):
                        nc.tensor.matmul(
                            acc[:],
                            lhs_of(c),
                            rT[c][:, sp * SPAN:(sp + 1) * SPAN],
                            start=False,
                            stop=(c == NCHUNK - 1),
                            skip_group_check=True,
                        )
                    nc.vector.tensor_copy(dst, acc[:])
                return [go_a, go_b]

            def q_proj_unit(sp, eblk):
                return _chain_units(
                    lambda c: wq_sb[c][:, eblk * P:(eblk + 1) * P], sp,
                    qT[eblk][:, sp * SPAN:(sp + 1) * SPAN])

            def k_proj_unit(sp):
                return _chain_units(
                    lambda c: wk_sb[c], sp,
                    kT[:, sp * SPAN:(sp + 1) * SPAN])

            def v_proj_unit(kt):
                def go():
                    acc = ps_f.tile([P, SPAN], F32, tag="f", name="vacc")
                    for c in range(NCHUNK):
                        nc.tensor.matmul(
                            acc[:, 0:128],
                            rT[c][:, kt * P:(kt + 1) * P],
                            wv_sb[c],
                            start=(c == 0),
                            stop=(c == NCHUNK - 1),
                        )
                    nc.vector.tensor_copy(vaug[kt][:, 0:64], acc[:, 0:64])
                    nc.vector.tensor_copy(vaug[kt][:, 65:129], acc[:, 64:128])
                return go

            def op_unit(sp, st):
                s0 = sp * SPAN + st * P
                cell = {}

                def go_dsp(dsp):
                    def go():
                        if dsp == 0:
                            cell['o'] = op.tile([P, D], BF16, tag="ost", name="osb")
                        o_sb = cell['o']
                        o_ps = ps_f.tile([P, SPAN], F32, tag="f", name="ops")
                        for ch in range(2):
                            nc.tensor.matmul(
                                o_ps[:],
                                zc[sp][ch][:, st * P:(st + 1) * P],
                                wo_sb[ch][:, dsp * SPAN:(dsp + 1) * SPAN],
                                start=(ch == 0),
                                stop=(ch == 1),
                            )
                        nc.vector.tensor_copy(
                            o_sb[:, dsp * SPAN:(dsp + 1) * SPAN], o_ps[:])
                        if dsp == 1:
                            nc.sync.dma_start(out_d[s0:s0 + P, :], o_sb[:])
                    return go
                return [go_dsp(0), go_dsp(1)]

            def pump(n, ops_ok=False):
                for _ in range(n):
                    if filler:
                        filler.pop(0)()
                    elif ops_ok and op_filler:
                        op_filler.pop(0)()
                    else:
                        break

            def proj_units(sp):
                u = k_proj_unit(sp) + q_proj_unit(sp, 0) + q_proj_unit(sp, 1)
                u += [v_proj_unit(kt) for kt in range(4 * sp, 4 * sp + 4)]
                return u

            # span 0 projections run up front
            for f in proj_units(0):
                f()

            for sp in range(NSPAN):
                q0 = sp * SPAN
                nkt = (q0 + SPAN) // P
                if sp + 1 < NSPAN:
                    filler.extend(proj_units(sp + 1))
                for ip in range(2):
                    u_ps = [ps_u.tile([65, SPAN], F32, tag="u", name=f"u{g}")
                            for g in range(2)]

                    def emit_av(b):
                        kt_, off_, w_, e_ = b
                        for g in range(2):
                            nc.tensor.matmul(
                                u_ps[g][0:65, off_:off_ + w_],
                                vaug[kt_][:, g * 65:(g + 1) * 65],
                                e_[:, g * 512 + off_:g * 512 + off_ + w_],
                                start=(kt_ == 0),
                                stop=(kt_ == nkt - 1),
                                skip_group_check=True,
                            )

                    pending = None
                    for kt in range(nkt):
                        k0 = kt * P
                        off = max(k0 - q0, 0)
                        w = SPAN - off
                        s_ps = ps_sc.tile([P, 2 * SPAN], F32, tag="sc",
                                          name="sps")
                        for g in range(2):
                            nc.tensor.matmul(
                                s_ps[:, g * 512 + off:g * 512 + off + w],
                                kT[g * 64:(g + 1) * 64, k0:k0 + P],
                                qT[ip][g * 64:(g + 1) * 64,
                                       q0 + off:q0 + off + w],
                                start=True,
                                stop=True,
                            )
                        e_sb = ep.tile([P, 2 * SPAN], BF16, tag="e", name="e")
                        nc.scalar.activation(
                            e_sb[:, off:2 * SPAN], s_ps[:, off:2 * SPAN],
                            EXP, scale=0.125,
                        )
                        if k0 >= q0:  # diagonal tile -> causal mask
                            for g in range(2):
                                nc.vector.tensor_mul(
                                    e_sb[:, g * 512 + off:g * 512 + off + P],
                                    e_sb[:, g * 512 + off:g * 512 + off + P],
                                    mask[:],
                                )
                        if debug and (sp, ip, kt) in ((0, 0, 0), (1, 0, 2)):
                            t = dbg_out(f"d_e_{sp}_{ip}_{kt}",
                                        [P, 2 * SPAN], BF16)
                            nc.sync.dma_start(t[:], e_sb[:])
                        if pending is not None:
                            emit_av(pending)
                        pending = (kt, off, w, e_sb)
                        pump(1, ops_ok=(sp == NSPAN - 1))
                    emit_av(pending)
                    if debug and sp == 0 and ip == 0:
                        for g in range(2):
                            us = pp.tile([65, SPAN], F32, tag=f"dbgu{g}")
                            nc.vector.tensor_copy(us[:], u_ps[g][:])
                            t = dbg_out(f"d_u_{g}", [65, SPAN], F32)
                            nc.sync.dma_start(t[:], us[:])

                    # normalize this pass -> z^T slabs
                    for g in range(2):
                        # standard-op copy remaps partition 64 -> 0; the
                        # custom-DVE reciprocal needs lane-aligned operands
                        row = mp.tile([1, SPAN], F32, tag="row", name="row")
                        nc.vector.tensor_copy(row[:], u_ps[g][64:65, :])
                        rec = mp.tile([1, SPAN], F32, tag="rec", name="rec")
                        nc.vector.reciprocal_approx_fast(rec[:], row[:])
                        bc = mp.tile([64, SPAN], F32, tag="bc", name="bc")
                        nc.gpsimd.partition_broadcast(bc[:], rec[:])
                        nc.vector.tensor_mul(
                            zc[sp][ip][g * 64:(g + 1) * 64, :],
                            u_ps[g][0:64, :],
                            bc[:],
                        )
                        if debug and sp == 0 and ip == 0:
                            rs = pp.tile([1, SPAN], F32, tag=f"dbgr{g}")
                            nc.vector.tensor_copy(rs[:], rec[:])
                            t = dbg_out(f"d_rec_{g}", [1, SPAN], F32)
                            nc.sync.dma_start(t[:], rs[:])
                if debug and sp == 0:
                    for i in range(2):
                        t = dbg_out(f"d_zc_{i}", [P, SPAN], BF16)
                        nc.sync.dma_start(t[:], zc[0][i][:])
                for st in range(4):
                    op_filler.extend(op_unit(sp, st))
            pump(len(filler) + len(op_filler), ops_ok=True)
            if debug:
                for nm, ap in (("d_mask", mask), ("d_kT", kT),
                               ("d_qT0", qT[0]), ("d_qT1", qT[1]),
                               ("d_va0", vaug[0]), ("d_va5", vaug[5])):
                    t = dbg_out(nm, list(ap.shape), BF16)
                    nc.sync.dma_start(t[:], ap[:])

    nc.finalize()
    return nc


def _pack_weights(wq4, wk2, wv2, wo4):
    """Pack per-core weight slices into the [128, 6144] bf16 layout the
    kernel expects: per d-chunk c, cols [c*512, c*512+512) = wk|wv|wq for
    rows c*128..c*128+127; wo chunk r at cols 4096 + r*1024."""
    bf16 = ml_dtypes.bfloat16
    wq = np.ascontiguousarray(wq4.reshape(D, 256))
    wk = np.ascontiguousarray(wk2.reshape(D, 128))
    wv = np.ascontiguousarray(wv2.reshape(D, 128))
    wo = np.ascontiguousarray(wo4.transpose(1, 0, 2).reshape(256, D))
    wp = np.zeros((P, 6144), np.float32)
    for c in range(NCHUNK):
        r = slice(c * P, (c + 1) * P)
        wp[:, c * 512:c * 512 + 128] = wk[r, :]
        wp[:, c * 512 + 128:c * 512 + 256] = wv[r, :]
        wp[:, c * 512 + 256:c * 512 + 512] = wq[r, :]
    wp[:, 4096:5120] = wo[0:128, :]
    wp[:, 5120:6144] = wo[128:256, :]
    return wp.astype(bf16)


def kernel(resid, W_Q, W_K, W_V, W_out, b_out):
    global LAST_RESULTS, _CACHED_NC
    resid = np.asarray(resid, np.float32)
    W_Q = np.asarray(W_Q, np.float32)
    W_K = np.asarray(W_K, np.float32)
    W_V = np.asarray(W_V, np.float32)
    W_out = np.asarray(W_out, np.float32)
    b_out = np.asarray(b_out, np.float32)
    bf16 = ml_dtypes.bfloat16

    if _CACHED_NC is None:
        _CACHED_NC = _build_program()
    nc = _CACHED_NC

    residT = [np.ascontiguousarray(resid[b].T).astype(bf16) for b in range(2)]
    in_maps = []
    for c in range(8):
        b, q = c // 4, c % 4
        # interleaved head order [h0, h2, h1, h3]: storage slot (g, i) holds
        # local head 2g+i -> qT[i]/zc[i] rows g*64 (see _build_program)
        heads = [4 * q, 4 * q + 2, 4 * q + 1, 4 * q + 3]
        groups = [2 * q, 2 * q + 1]
        in_maps.append({
            "resid_t": residT[b],
            "wpack": _pack_weights(W_Q[:, heads, :], W_K[:, groups, :],
                                   W_V[:, groups, :], W_out[:, heads, :]),
        })

    res = run_bass_kernel_spmd(nc, in_maps, core_ids=list(range(8)))
    LAST_RESULTS = res

    out = np.zeros((2, S, D), np.float32)
    for c in range(8):
        out[c // 4] += np.asarray(res.results[c]["out"], np.float32)
    out += b_out
    return out
